# revision 48
# baseline (speedup 1.0000x reference)
"""Trainium2 Bass kernel for retrieval-knn attention classifier (nn_MA_51866025067137).

Strategy (8 NeuronCores):
  Phase 1 — memory_keys sharded along N (12800 keys/core, padded 100000->102400
  with zero rows).  Keys are L2-normalized and quantized to fp8e4m3 on host;
  each core computes ranking sims for all 256 queries against its shard with
  fp8 DoubleRow matmuls (2 rows/partition, 0.5 cyc/col) and the ACT engine
  evicts each PSUM chunk as Relu(sim - tau_b) in fp8 — a per-key candidate
  flag.  tau_b is a per-query statistical threshold (mu + z*sigma, estimated
  host-side from a key sample) tuned to flag ~50 keys/core/query.  The flag
  map (1 B/key) is DMA'd out; the host scans nonzeros, re-scores the ~400
  candidates per query exactly in fp32, and takes the global top-32.
  Phase 2 — batch sharded (32 queries/core): memory-attention module + classifier
  entirely in bf16 matmuls from a single weight blob: scores are computed in
  transposed layout (no DRAM bounce), attended^T is computed directly via
  per-tile matmuls against the softmax block weights (no PE transposes), and
  the 1/den normalization is folded into a final fused multiply-add.
"""

import numpy as np
import ml_dtypes

import concourse.bacc as bacc
import concourse.mybir as mybir
from concourse.tile import TileContext
from concourse.bass_utils import run_bass_kernel_spmd

# problem dims (hardcoded per harness contract)
B, N, D = 256, 100000, 512
A, C, K = 256, 100, 32
NC_CORES = 8
NPAD = 102400             # 8 * 12800
SHARD = NPAD // NC_CORES  # 12800
CHUNK = 512               # keys per matmul chunk
NCHUNK = SHARD // CHUNK   # 25
BROWS = B // NC_CORES     # 32 rows per core in phase 2
NCD = BROWS * K           # 1024
ZTHRESH = 2.62            # flag-rate z-score (~4.4e-3 -> ~56 flags/core/row)
NSAMP = 2048              # host-side sample size for per-row sim stats

f32 = mybir.dt.float32
bf16 = mybir.dt.bfloat16
f8 = mybir.dt.float8e4
u32 = mybir.dt.uint32
F8NP = ml_dtypes.float8_e4m3
BF16NP = ml_dtypes.bfloat16

_PH1 = None
_PH2 = None

# weight-blob column offsets (phase 2; Wm ships as fp8, qproj/outq computed
# on host)
WS_OFF = 0                # 2 x [128, 1]
QPB_OFF = 2               # [32, 256] = WM_SCALE * q @ Wq (rows 0-31)
WBA_COLS = 258            # first dma piece
WC_OFF = 258              # 4 x [128, 100] (att half of Wc only)
WB_COLS = 658
WM_SCALE = 16.0           # wm8/qpb pre-scale so fp8 stays in normal range


def _build_phase1():
    nc = bacc.Bacc("TRN2", target_bir_lowering=False)
    k8_d = nc.dram_tensor("k8", [NCHUNK, 128, 4 * CHUNK], f8, kind="ExternalInput")
    q8_d = nc.dram_tensor("q8", [128, 1024], f8, kind="ExternalInput")
    tau_d = nc.dram_tensor("tau", [128, 2], f32, kind="ExternalInput")
    fl_d = nc.dram_tensor("fl", [2, 128, SHARD], f8, kind="ExternalOutput")

    # chunk pairs: one key DMA + one [128,1024] PSUM per pair per qt; evicts
    # split across ACT and DVE; flags live in per-piece tiles so dump DMAs
    # never block later writes.  Last piece is a single pair (short tail).
    NPAIR = 13                              # 12 pairs of 2 chunks + 1 single
    pieces = [(0, 3), (3, 6), (6, 9), (9, 11), (11, 12), (12, 13)]

    with TileContext(nc) as tc:
        with (
            tc.tile_pool(name="qp", bufs=1) as qp,
            tc.tile_pool(name="keys", bufs=6) as keyp,
            tc.tile_pool(name="fl", bufs=1) as flp,
            tc.tile_pool(name="psum", bufs=2, space="PSUM") as psump,
        ):
            # first key DMA dispatched first: its transfer gates the pipeline
            kt0 = keyp.tile([128, 4 * 2 * CHUNK], f8, tag="kt")
            nc.sync.dma_start(
                out=kt0[:].rearrange("p (n x) -> p n x", n=2),
                in_=k8_d[0:2, :, :].rearrange("n p x -> p n x"))
            q8 = qp.tile([128, 1024], f8, tag="q8")
            nc.sync.dma_start(out=q8[:], in_=q8_d[:, :])
            tau = qp.tile([128, 2], f32, tag="tau")
            nc.sync.dma_start(out=tau[:], in_=tau_d[:, :])
            zeros = qp.tile([128, 2 * CHUNK], f32, tag="zeros")
            nc.vector.memset(zeros[:], 0.0)
            q8v = q8[:].rearrange("p (mc two b) -> p mc two b", mc=2, two=2)

            def pair_w(pr):
                return (2 if pr < 12 else 1) * CHUNK

            fl = {}
            for qt in range(2):
                for pi, (p0, p1) in enumerate(pieces):
                    fl[qt, pi] = flp.tile(
                        [128, sum(pair_w(p) for p in range(p0, p1))], f8,
                        tag=f"fl{qt}_{pi}", name=f"flt{qt}_{pi}")

            for pr in range(NPAIR):
                nch = 2 if pr < 12 else 1            # chunks in this pair
                w = nch * CHUNK
                if pr == 0:
                    kt = kt0
                else:
                    kt = keyp.tile([128, 4 * 2 * CHUNK], f8, tag="kt")
                    nc.sync.dma_start(
                        out=kt[:, :nch * 4 * CHUNK].rearrange(
                            "p (n x) -> p n x", n=nch),
                        in_=k8_d[2 * pr:2 * pr + nch, :, :].rearrange(
                            "n p x -> p n x"))
                pi = next(i for i, (p0, p1) in enumerate(pieces) if pr < p1)
                p0 = pieces[pi][0]
                off = (pr - p0) * 2 * CHUNK
                for qt in range(2):
                    ps = psump.tile([128, 2 * CHUNK], f32, tag=f"ps{qt}")
                    for ci in range(nch):
                        ktv = kt[:, ci * 4 * CHUNK:(ci + 1) * 4 * CHUNK].rearrange(
                            "p (mc two j) -> p mc two j", mc=2, two=2)
                        for mc in range(2):
                            nc.tensor.matmul(
                                ps[:, ci * CHUNK:(ci + 1) * CHUNK],
                                lhsT=q8v[:, mc, :, qt * 128:(qt + 1) * 128],
                                rhs=ktv[:, mc, :, :],
                                start=(mc == 0), stop=(mc == 1),
                                perf_mode=mybir.MatmulPerfMode.DoubleRow,
                            )
                    dst = fl[qt, pi][:, off:off + w]
                    if (pr % 3) == 2:
                        # DVE evict: max(ps + tau, 0) -> fp8 flags
                        nc.vector.scalar_tensor_tensor(
                            out=dst, in0=ps[:, :w], scalar=tau[:, qt:qt + 1],
                            in1=zeros[:, :w],
                            op0=mybir.AluOpType.add, op1=mybir.AluOpType.max)
                    else:
                        nc.scalar.activation(
                            dst, ps[:, :w],
                            mybir.ActivationFunctionType.Relu,
                            bias=tau[:, qt:qt + 1])
                if pr + 1 == pieces[pi][1]:          # piece complete -> dump
                    # qt0 on the idle Pool sequencer (inline) so its sem-wait
                    # never head-of-line-blocks SP's key-DMA dispatches
                    c0 = pieces[pi][0] * 2 * CHUNK
                    nc.gpsimd.dma_start(
                        out=fl_d[0, :, c0:c0 + fl[0, pi].shape[1]],
                        in_=fl[0, pi][:])
            # qt1 dumps on SP after every key dispatch is already emitted
            for pi, (p0, p1) in enumerate(pieces):
                c0 = p0 * 2 * CHUNK
                nc.sync.dma_start(
                    out=fl_d[1, :, c0:c0 + fl[1, pi].shape[1]],
                    in_=fl[1, pi][:])
    nc.finalize()
    return nc


def _build_phase2():
    nc = bacc.Bacc("TRN2", target_bir_lowering=False)
    wb_d = nc.dram_tensor("wb", [128, WB_COLS], bf16, kind="ExternalInput")
    bmat_d = nc.dram_tensor("bmat", [BROWS, NCD], bf16, kind="ExternalInput")
    wm8_d = nc.dram_tensor("wm8", [128, 1024], f8, kind="ExternalInput")
    bqm_d = nc.dram_tensor("bqm", [128, 2], f32, kind="ExternalInput")
    outq_d = nc.dram_tensor("outq", [BROWS, C], f32, kind="ExternalInput")
    knnT8_d = nc.dram_tensor("knnT8", [128, 4 * NCD], f8, kind="ExternalInput")
    knn_d = nc.dram_tensor("knn", [128, 8 * D], bf16, kind="ExternalInput")
    out_d = nc.dram_tensor("out", [BROWS, C], f32, kind="ExternalOutput")

    with TileContext(nc) as tc:
        with (
            tc.tile_pool(name="sb", bufs=1) as sb,
            tc.tile_pool(name="kp_ps", bufs=1, space="PSUM") as kpp,
            tc.tile_pool(name="ps1", bufs=1, space="PSUM") as ps1,
        ):
            # ---- constants (off critical path) ----
            # maskwide [128, 60]: M[p, c] = 1 iff c == p//32 + 28
            mw = sb.tile([128, 60], bf16, tag="mw")
            nc.vector.memset(mw[:], 1.0)
            nc.gpsimd.affine_select(out=mw[:], in_=mw[:],
                                    compare_op=mybir.AluOpType.is_ge, fill=0.0,
                                    base=896, pattern=[[-32, 60]],
                                    channel_multiplier=1)
            nc.gpsimd.affine_select(out=mw[:], in_=mw[:],
                                    compare_op=mybir.AluOpType.is_ge, fill=0.0,
                                    base=-865, pattern=[[32, 60]],
                                    channel_multiplier=-1)
            ones1 = sb.tile([128, 1], bf16, tag="ones1")
            nc.vector.memset(ones1[:], 1.0)

            # ---- inputs (order = criticality: wbA, wm8, knnT8, knn, Wc) ----
            wb = sb.tile([128, WB_COLS], bf16, tag="wb")
            wm8 = sb.tile([128, 1024], f8, tag="wm8")
            bqm = sb.tile([128, 2], f32, tag="bqm")
            outq = sb.tile([BROWS, C], f32, tag="outq_sb")
            knnT8 = sb.tile([128, 4 * NCD], f8, tag="knnT8")
            knn = sb.tile([128, 8 * D], bf16, tag="knn")
            Bmat = sb.tile([BROWS, NCD], bf16, tag="Bmat")
            nc.sync.dma_start(out=wb[:, :WBA_COLS], in_=wb_d[:, :WBA_COLS])
            nc.sync.dma_start(out=Bmat[:], in_=bmat_d[:, :])
            nc.sync.dma_start(out=wm8[:], in_=wm8_d[:, :])
            wm8v = wm8[:].rearrange("p (mc two a) -> p mc two a", mc=2, two=2)
            kt8v = knnT8[:].rearrange("p (mc two j) -> p mc two j", mc=2, two=2)
            kt8v_d = knnT8_d[:, :].rearrange("p (mc two j) -> p mc two j",
                                             mc=2, two=2)
            # knnT8 split by cand-half so tanh/scores pipeline per half
            for half in range(2):
                sl = slice(half * 512, (half + 1) * 512)
                nc.sync.dma_start(out=kt8v[:, :, :, sl], in_=kt8v_d[:, :, :, sl])
            nc.sync.dma_start(out=bqm[:], in_=bqm_d[:, :])
            nc.sync.dma_start(out=outq[:], in_=outq_d[:, :])
            nc.sync.dma_start(out=knn[:], in_=knn_d[:, :])
            nc.sync.dma_start(out=wb[:, WBA_COLS:], in_=wb_d[:, WBA_COLS:])
            qpb = wb[:BROWS, QPB_OFF:QPB_OFF + A]     # host-computed, scaled

            def wc(m):
                off = WC_OFF + m * 100
                return wb[:, off:off + 100]

            # ---- kp[at] = WM_SCALE*(knn @ Wm + qproj bcast); hT = tanh(kp/S
            # + bqm).  fp8 DoubleRow, mc-outer: each round gated by its own
            # knnT8 piece DMA so the PE never builds a queued backlog ----
            hT = [sb.tile([128, NCD], bf16, tag=f"hT{at}", name=f"hTt{at}")
                  for at in range(2)]
            kp = [kpp.tile([128, NCD], f32, tag=f"kp{at}", name=f"kpt{at}")
                  for at in range(2)]
            # qproj fold goes FIRST (needs only wbA+Bmat -> overlaps knnT8 DMA)
            for at in range(2):
                for half in range(2):
                    sl = slice(half * 512, (half + 1) * 512)
                    off = QPB_OFF + at * 128
                    nc.tensor.matmul(
                        kp[at][:, sl], lhsT=wb[:BROWS, off:off + 128],
                        rhs=Bmat[:, sl], start=True, stop=False,
                        skip_group_check=True)
            sc_ps = ps1.tile([128, 8], f32, tag="small2")
            for half in range(2):
                sl = slice(half * 512, (half + 1) * 512)
                for mc in range(2):
                    for at in range(2):
                        nc.tensor.matmul(
                            kp[at][:, sl],
                            lhsT=wm8v[:, mc, :, at * 128:(at + 1) * 128],
                            rhs=kt8v[:, mc, :, sl],
                            start=False, stop=(mc == 1),
                            perf_mode=mybir.MatmulPerfMode.DoubleRow,
                            skip_group_check=True)
                for at in range(2):
                    nc.scalar.activation(hT[at][:, sl], kp[at][:, sl],
                                         mybir.ActivationFunctionType.Tanh,
                                         bias=bqm[:, at:at + 1],
                                         scale=1.0 / WM_SCALE)
                # scoresT for this half's cand tiles
                for t in range(half * 4, half * 4 + 4):
                    for at in range(2):
                        nc.tensor.matmul(
                            sc_ps[:, t:t + 1],
                            lhsT=hT[at][:, t * 128:(t + 1) * 128],
                            rhs=wb[:, WS_OFF + at:WS_OFF + at + 1],
                            start=(at == 0), stop=(at == 1))
            e_sb = sb.tile([128, 8], f32, tag="e_sb")
            nc.scalar.activation(e_sb[:], sc_ps[:],
                                 mybir.ActivationFunctionType.Exp)

            # ---- block softmax weights w2 [128, 256]: w2[p, t*32+b] ----
            w2 = sb.tile([128, 256], bf16, tag="w2")
            for t in range(8):
                nc.vector.tensor_scalar_mul(
                    w2[:, t * 32:(t + 1) * 32],
                    mw[:, 28 - 4 * t:60 - 4 * t],
                    e_sb[:, t:t + 1])

            # ---- den, attT (unscaled), classifier ----
            den_t = ps1.tile([128, 8], f32, tag="small2")
            den_ps = den_t[:BROWS, :1]
            for t in range(8):
                nc.tensor.matmul(den_ps, lhsT=w2[:, t * 32:(t + 1) * 32],
                                 rhs=ones1[:], start=(t == 0), stop=(t == 7))
            rden = sb.tile([BROWS, 1], f32, tag="rden")
            nc.vector.reciprocal(rden[:], den_ps)

            attT_ps = ps1.tile([128, 128], f32, tag="attT")
            for dc in range(4):
                for t in range(8):
                    nc.tensor.matmul(
                        attT_ps[:, dc * 32:(dc + 1) * 32],
                        lhsT=knn[:, t * D + dc * 128:t * D + (dc + 1) * 128],
                        rhs=w2[:, t * 32:(t + 1) * 32],
                        start=(t == 0), stop=(t == 7))
            attT = sb.tile([128, 128], bf16, tag="attT_sb")
            nc.scalar.copy(out=attT[:], in_=attT_ps[:])

            outa_ps = ps1.tile([BROWS, C], f32, tag="outa")
            for dc in range(4):
                nc.tensor.matmul(outa_ps[:], lhsT=attT[:, dc * 32:(dc + 1) * 32],
                                 rhs=wc(dc), start=(dc == 0), stop=(dc == 3))

            # out = outa * rden + outq  (fold 1/den at the end)
            out_sb = sb.tile([BROWS, C], f32, tag="out_sb")
            nc.vector.scalar_tensor_tensor(
                out=out_sb[:], in0=outa_ps[:], scalar=rden[:], in1=outq[:],
                op0=mybir.AluOpType.mult, op1=mybir.AluOpType.add)
            nc.sync.dma_start(out=out_d[:, :], in_=out_sb[:])
    nc.finalize()
    return nc


def _phase1_nc():
    global _PH1
    if _PH1 is None:
        _PH1 = _build_phase1()
    return _PH1


def _phase2_nc():
    global _PH2
    if _PH2 is None:
        _PH2 = _build_phase2()
    return _PH2


def kernel(query_feat, memory_keys, Wq, bq, Wm, bm, Ws, bs, Wc, bc):
    query_feat = np.asarray(query_feat, np.float32)
    memory_keys = np.asarray(memory_keys, np.float32)

    # ---- host prep: normalize keys, quantize to fp8, DoubleRow layout ----
    kn = np.sqrt((memory_keys ** 2).sum(axis=1))
    khat = memory_keys * (1.0 / kn)[:, None]
    khat_pad = np.zeros((NPAD, D), np.float32)
    khat_pad[:N] = khat
    k8 = khat_pad.astype(F8NP)

    q32 = np.maximum(query_feat, 0)
    q8 = q32.astype(F8NP)

    # per-row flag threshold tau_b = mu_b + z * sigma_b of the fp8 sims,
    # estimated from a uniform key sample (exact same arrays the PE sees)
    k8f = k8[:N].astype(np.float32)
    q8f = q8.astype(np.float32)
    samp = k8f[:: N // NSAMP][:NSAMP]
    sims_s = q8f @ samp.T                              # [B, NSAMP]
    mu = sims_s.mean(axis=1)
    sig = sims_s.std(axis=1)
    tau = (mu + ZTHRESH * sig).astype(np.float32)      # [B]
    # bias for ACT: Relu(sim + bias), bias = -tau, laid out [128, 2(qt)]
    bias = (-tau).reshape(2, 128).T.copy()             # [128, 2]

    # q8 DoubleRow layout: [128p, (mc two b)]
    q8arr = np.ascontiguousarray(
        q8.T.reshape(2, 2, 128, B).transpose(2, 0, 1, 3)).reshape(128, 1024)

    ph1 = _phase1_nc()
    in_maps = []
    for c in range(NC_CORES):
        sh = k8[c * SHARD:(c + 1) * SHARD]             # [12800, 512]
        arr = np.ascontiguousarray(
            sh.reshape(NCHUNK, CHUNK, 2, 2, 128).transpose(0, 4, 2, 3, 1)
        ).reshape(NCHUNK, 128, 4 * CHUNK)
        in_maps.append({"k8": arr, "q8": q8arr, "tau": bias})
    res1 = run_bass_kernel_spmd(ph1, in_maps, core_ids=list(range(NC_CORES)))

    # ---- host: scan flags, exact re-score, global top-32 ----
    flags = np.empty((B, NPAD), np.uint8)
    for c in range(NC_CORES):
        fl = res1.results[c]["fl"].view(np.uint8)      # [2, 128, SHARD]
        flags[:, c * SHARD:(c + 1) * SHARD] = fl.reshape(B, SHARD)
    flags &= 0x7F                                      # ignore sign bit of -0
    rows, cols = np.nonzero(flags)
    cnt = np.bincount(rows, minlength=B)
    Mx = max(int(cnt.max()), K)
    idxpad = np.zeros((B, Mx), np.int64)
    mask = np.arange(Mx)[None, :] < cnt[:, None]
    idxpad[mask] = cols
    ok = mask & (idxpad < N)

    qn = np.sqrt((q32 ** 2).sum(axis=1))
    safe = np.minimum(idxpad, N - 1)
    cand_keys = memory_keys[safe]                      # [B, Mx, D]
    dots = np.einsum("bd,bmd->bm", q32, cand_keys, optimize=True)
    cos = dots / np.maximum(qn[:, None] * kn[safe], np.float32(1e-8))
    cos[~ok] = -np.inf

    short = np.nonzero(ok.sum(axis=1) < K)[0]
    if short.size:                                     # statistical fallback
        sims_full = q32[short] @ memory_keys.T
        cos_full = sims_full / np.maximum(
            qn[short, None] * kn[None, :], np.float32(1e-8))
        top_f = np.argsort(-cos_full, axis=1, kind="stable")[:, :K]
    order = np.argsort(-cos, axis=1, kind="stable")[:, :K]
    top_idx = np.take_along_axis(safe, order, axis=1)  # [B, K]
    if short.size:
        top_idx[short] = top_f
    knn = memory_keys[top_idx]                         # [B, K, D]

    # ---- phase 2 (batch sharded, bf16 blob + fp8 scores path) ----
    ph2 = _phase2_nc()
    Wq_a = np.asarray(Wq, np.float32)
    Wm_a = np.asarray(Wm, np.float32)
    Ws_a = np.asarray(Ws, np.float32).reshape(A)
    Wc_a = np.asarray(Wc, np.float32)
    bqm = (np.asarray(bq, np.float32) + np.asarray(bm, np.float32))
    bqm_arr = np.ascontiguousarray(bqm.reshape(2, 128).T)          # [128, 2]

    wb = np.zeros((128, WB_COLS), np.float32)
    wb[:, WS_OFF:WS_OFF + 2] = Ws_a.reshape(2, 128).T
    wb[:, WC_OFF:WC_OFF + 400] = (
        Wc_a[D:].reshape(4, 128, C).transpose(1, 0, 2).reshape(128, 400))
    # wm8 [128p, (mc two a)] = WM_SCALE * Wm[mc*256+two*128+p, a]
    wm8_arr = np.ascontiguousarray(
        (WM_SCALE * Wm_a).reshape(2, 2, 128, A).transpose(2, 0, 1, 3)
    ).reshape(128, 1024).astype(F8NP)
    qproj_all = (q32 @ Wq_a) * WM_SCALE                # [B, A] host-side
    outq_all = q32 @ Wc_a[:D]                          # [B, C] host-side, exact
    # bmat [32, 1024]: B[b, cand] = 1 iff cand // 32 == b
    bmat_arr = (np.arange(NCD)[None, :] // K == np.arange(BROWS)[:, None]
                ).astype(BF16NP)

    in_maps2 = []
    for c in range(NC_CORES):
        rows2 = slice(c * BROWS, (c + 1) * BROWS)
        wb_c = wb.copy()
        wb_c[:BROWS, QPB_OFF:QPB_OFF + A] = qproj_all[rows2]
        knn_c = knn[rows2].reshape(NCD, D)             # [1024, 512]
        # knnT8 [128p, (mc two cand)] = knn_c[cand, mc*256+two*128+p]
        knnT8_c = np.ascontiguousarray(
            knn_c.T.reshape(2, 2, 128, NCD).transpose(2, 0, 1, 3)
        ).reshape(128, 4 * NCD).astype(F8NP)
        knn_l = np.ascontiguousarray(
            knn_c.reshape(8, 128, D).transpose(1, 0, 2).reshape(128, 8 * D))
        in_maps2.append({
            "wb": wb_c.astype(BF16NP),
            "bmat": bmat_arr,
            "wm8": wm8_arr,
            "bqm": bqm_arr,
            "outq": np.ascontiguousarray(outq_all[rows2]),
            "knnT8": knnT8_c,
            "knn": knn_l.astype(BF16NP),
        })
    res2 = run_bass_kernel_spmd(ph2, in_maps2, core_ids=list(range(NC_CORES)))
    out = np.concatenate([res2.results[c]["out"] for c in range(NC_CORES)], axis=0)
    return (out + np.asarray(bc, np.float32)[None, :]).astype(np.float32)


# revision 51
# speedup vs baseline: 1.0468x; 1.0468x over previous
"""Trainium2 Bass kernel for retrieval-knn attention classifier (nn_MA_51866025067137).

Strategy (8 NeuronCores):
  Phase 1 — memory_keys sharded along N (12800 keys/core, padded 100000->102400
  with zero rows).  Keys are L2-normalized and quantized to fp8e4m3 on host;
  each core computes ranking sims for all 256 queries against its shard with
  fp8 DoubleRow matmuls (2 rows/partition, 0.5 cyc/col) and the ACT engine
  evicts each PSUM chunk as Relu(sim - tau_b) in fp8 — a per-key candidate
  flag.  tau_b is a per-query statistical threshold (mu + z*sigma, estimated
  host-side from a key sample) tuned to flag ~50 keys/core/query.  The flag
  map (1 B/key) is DMA'd out; the host scans nonzeros, re-scores the ~400
  candidates per query exactly in fp32, and takes the global top-32.
  Phase 2 — batch sharded (32 queries/core): memory-attention module + classifier
  entirely in bf16 matmuls from a single weight blob: scores are computed in
  transposed layout (no DRAM bounce), attended^T is computed directly via
  per-tile matmuls against the softmax block weights (no PE transposes), and
  the 1/den normalization is folded into a final fused multiply-add.
"""

import numpy as np
import ml_dtypes

import concourse.bacc as bacc
import concourse.mybir as mybir
from concourse.tile import TileContext
from concourse.bass_utils import run_bass_kernel_spmd

# problem dims (hardcoded per harness contract)
B, N, D = 256, 100000, 512
A, C, K = 256, 100, 32
NC_CORES = 8
NPAD = 102400             # 8 * 12800
SHARD = NPAD // NC_CORES  # 12800
CHUNK = 512               # keys per matmul chunk
NCHUNK = SHARD // CHUNK   # 25
BROWS = B // NC_CORES     # 32 rows per core in phase 2
NCD = BROWS * K           # 1024
ZTHRESH = 2.62            # flag-rate z-score (~4.4e-3 -> ~56 flags/core/row)
NSAMP = 2048              # host-side sample size for per-row sim stats

f32 = mybir.dt.float32
bf16 = mybir.dt.bfloat16
f8 = mybir.dt.float8e4
u32 = mybir.dt.uint32
F8NP = ml_dtypes.float8_e4m3
BF16NP = ml_dtypes.bfloat16

_PH1 = None
_PH2 = None

# weight-blob column offsets (phase 2; Wm ships as fp8, qproj/outq computed
# on host)
WS_OFF = 0                # 2 x [128, 1]
QPB_OFF = 2               # [32, 256] = WM_SCALE * q @ Wq (rows 0-31)
WBA_COLS = 258            # first dma piece
WC_OFF = 258              # 4 x [128, 100] (att half of Wc only)
WB_COLS = 658
WM_SCALE = 16.0           # wm8/qpb pre-scale so fp8 stays in normal range


def _build_phase1():
    nc = bacc.Bacc("TRN2", target_bir_lowering=False)
    k8_d = nc.dram_tensor("k8", [NCHUNK, 128, 4 * CHUNK], f8, kind="ExternalInput")
    q8_d = nc.dram_tensor("q8", [128, 1024], f8, kind="ExternalInput")
    tau_d = nc.dram_tensor("tau", [128, 2], f32, kind="ExternalInput")
    fl_d = nc.dram_tensor("fl", [2, 128, SHARD], f8, kind="ExternalOutput")

    # chunk pairs: one key DMA + one [128,1024] PSUM per pair per qt; evicts
    # split across ACT and DVE; flags live in per-piece tiles so dump DMAs
    # never block later writes.  Last piece is a single pair (short tail).
    NPAIR = 13                              # 12 pairs of 2 chunks + 1 single
    pieces = [(0, 3), (3, 6), (6, 9), (9, 11), (11, 12), (12, 13)]

    with TileContext(nc) as tc:
        with (
            tc.tile_pool(name="qp", bufs=1) as qp,
            tc.tile_pool(name="keys", bufs=6) as keyp,
            tc.tile_pool(name="fl", bufs=1) as flp,
            tc.tile_pool(name="psum", bufs=2, space="PSUM") as psump,
        ):
            # first key DMA dispatched first: its transfer gates the pipeline
            kt0 = keyp.tile([128, 4 * 2 * CHUNK], f8, tag="kt")
            nc.sync.dma_start(
                out=kt0[:].rearrange("p (n x) -> p n x", n=2),
                in_=k8_d[0:2, :, :].rearrange("n p x -> p n x"))
            q8 = qp.tile([128, 1024], f8, tag="q8")
            nc.sync.dma_start(out=q8[:], in_=q8_d[:, :])
            tau = qp.tile([128, 2], f32, tag="tau")
            nc.sync.dma_start(out=tau[:], in_=tau_d[:, :])
            zeros = qp.tile([128, 2 * CHUNK], f32, tag="zeros")
            nc.vector.memset(zeros[:], 0.0)
            q8v = q8[:].rearrange("p (mc two b) -> p mc two b", mc=2, two=2)

            def pair_w(pr):
                return (2 if pr < 12 else 1) * CHUNK

            fl = {}
            for qt in range(2):
                for pi, (p0, p1) in enumerate(pieces):
                    fl[qt, pi] = flp.tile(
                        [128, sum(pair_w(p) for p in range(p0, p1))], f8,
                        tag=f"fl{qt}_{pi}", name=f"flt{qt}_{pi}")

            for pr in range(NPAIR):
                nch = 2 if pr < 12 else 1            # chunks in this pair
                w = nch * CHUNK
                if pr == 0:
                    kt = kt0
                else:
                    kt = keyp.tile([128, 4 * 2 * CHUNK], f8, tag="kt")
                    nc.sync.dma_start(
                        out=kt[:, :nch * 4 * CHUNK].rearrange(
                            "p (n x) -> p n x", n=nch),
                        in_=k8_d[2 * pr:2 * pr + nch, :, :].rearrange(
                            "n p x -> p n x"))
                pi = next(i for i, (p0, p1) in enumerate(pieces) if pr < p1)
                p0 = pieces[pi][0]
                off = (pr - p0) * 2 * CHUNK
                for qt in range(2):
                    ps = psump.tile([128, 2 * CHUNK], f32, tag=f"ps{qt}")
                    for ci in range(nch):
                        ktv = kt[:, ci * 4 * CHUNK:(ci + 1) * 4 * CHUNK].rearrange(
                            "p (mc two j) -> p mc two j", mc=2, two=2)
                        for mc in range(2):
                            nc.tensor.matmul(
                                ps[:, ci * CHUNK:(ci + 1) * CHUNK],
                                lhsT=q8v[:, mc, :, qt * 128:(qt + 1) * 128],
                                rhs=ktv[:, mc, :, :],
                                start=(mc == 0), stop=(mc == 1),
                                perf_mode=mybir.MatmulPerfMode.DoubleRow,
                            )
                    dst = fl[qt, pi][:, off:off + w]
                    if (pr % 3) == 2:
                        # DVE evict: max(ps + tau, 0) -> fp8 flags
                        nc.vector.scalar_tensor_tensor(
                            out=dst, in0=ps[:, :w], scalar=tau[:, qt:qt + 1],
                            in1=zeros[:, :w],
                            op0=mybir.AluOpType.add, op1=mybir.AluOpType.max)
                    else:
                        nc.scalar.activation(
                            dst, ps[:, :w],
                            mybir.ActivationFunctionType.Relu,
                            bias=tau[:, qt:qt + 1])
                if pr + 1 == pieces[pi][1]:          # piece complete -> dump
                    # qt0 on the idle Pool sequencer (inline) so its sem-wait
                    # never head-of-line-blocks SP's key-DMA dispatches
                    c0 = pieces[pi][0] * 2 * CHUNK
                    nc.gpsimd.dma_start(
                        out=fl_d[0, :, c0:c0 + fl[0, pi].shape[1]],
                        in_=fl[0, pi][:])
            # qt1 dumps on SP after every key dispatch is already emitted
            for pi, (p0, p1) in enumerate(pieces):
                c0 = p0 * 2 * CHUNK
                nc.sync.dma_start(
                    out=fl_d[1, :, c0:c0 + fl[1, pi].shape[1]],
                    in_=fl[1, pi][:])
    nc.finalize()
    return nc


def _build_phase2():
    nc = bacc.Bacc("TRN2", target_bir_lowering=False)
    wb_d = nc.dram_tensor("wb", [128, WB_COLS], bf16, kind="ExternalInput")
    bmat_d = nc.dram_tensor("bmat", [BROWS, NCD], bf16, kind="ExternalInput")
    wm8_d = nc.dram_tensor("wm8", [128, 1024], f8, kind="ExternalInput")
    bqm_d = nc.dram_tensor("bqm", [128, 2], f32, kind="ExternalInput")
    outq_d = nc.dram_tensor("outq", [BROWS, C], f32, kind="ExternalInput")
    knnT8_d = nc.dram_tensor("knnT8", [128, 4 * NCD], f8, kind="ExternalInput")
    knn_d = nc.dram_tensor("knn", [128, 8 * D], bf16, kind="ExternalInput")
    out_d = nc.dram_tensor("out", [BROWS, C], f32, kind="ExternalOutput")

    with TileContext(nc) as tc:
        with (
            tc.tile_pool(name="sb", bufs=1) as sb,
            tc.tile_pool(name="kp_ps", bufs=1, space="PSUM") as kpp,
            tc.tile_pool(name="ps1", bufs=1, space="PSUM") as ps1,
        ):
            # ---- constants (off critical path) ----
            # maskwide [128, 60]: M[p, c] = 1 iff c == p//32 + 28
            mw = sb.tile([128, 60], bf16, tag="mw")
            nc.vector.memset(mw[:], 1.0)
            nc.gpsimd.affine_select(out=mw[:], in_=mw[:],
                                    compare_op=mybir.AluOpType.is_ge, fill=0.0,
                                    base=896, pattern=[[-32, 60]],
                                    channel_multiplier=1)
            nc.gpsimd.affine_select(out=mw[:], in_=mw[:],
                                    compare_op=mybir.AluOpType.is_ge, fill=0.0,
                                    base=-865, pattern=[[32, 60]],
                                    channel_multiplier=-1)
            ones1 = sb.tile([128, 1], bf16, tag="ones1")
            nc.vector.memset(ones1[:], 1.0)
            # trigger the ACT function-table load early (off critical path)
            dummy = sb.tile([1, 1], f32, tag="dummy")
            nc.vector.memset(dummy[:], 0.0)
            nc.scalar.activation(dummy[:], dummy[:],
                                 mybir.ActivationFunctionType.Tanh)

            # ---- inputs (order = criticality: wbA, wm8, knnT8, knn, Wc) ----
            wb = sb.tile([128, WB_COLS], bf16, tag="wb")
            wm8 = sb.tile([128, 1024], f8, tag="wm8")
            bqm = sb.tile([128, 2], f32, tag="bqm")
            outq = sb.tile([BROWS, C], f32, tag="outq_sb")
            knnT8 = sb.tile([128, 4 * NCD], f8, tag="knnT8")
            knn = sb.tile([128, 8 * D], bf16, tag="knn")
            # input dispatches spread across SP/DVE sequencers so the 650ns
            # per-dispatch serialization doesn't delay the early transfers
            Bmat = sb.tile([BROWS, NCD], bf16, tag="Bmat")
            nc.sync.dma_start(out=wb[:, :WBA_COLS], in_=wb_d[:, :WBA_COLS])
            nc.scalar.dma_start(out=Bmat[:], in_=bmat_d[:, :])
            nc.scalar.dma_start(out=wm8[:], in_=wm8_d[:, :])
            wm8v = wm8[:].rearrange("p (mc two a) -> p mc two a", mc=2, two=2)
            kt8v = knnT8[:].rearrange("p (mc two j) -> p mc two j", mc=2, two=2)
            kt8v_d = knnT8_d[:, :].rearrange("p (mc two j) -> p mc two j",
                                             mc=2, two=2)
            # knnT8 split by cand-half so tanh/scores pipeline per half
            for half in range(2):
                sl = slice(half * 512, (half + 1) * 512)
                nc.sync.dma_start(out=kt8v[:, :, :, sl], in_=kt8v_d[:, :, :, sl])
            nc.gpsimd.dma_start(out=bqm[:], in_=bqm_d[:, :])
            nc.gpsimd.dma_start(out=outq[:], in_=outq_d[:, :])
            nc.sync.dma_start(out=knn[:], in_=knn_d[:, :])
            nc.sync.dma_start(out=wb[:, WBA_COLS:], in_=wb_d[:, WBA_COLS:])
            qpb = wb[:BROWS, QPB_OFF:QPB_OFF + A]     # host-computed, scaled

            def wc(m):
                off = WC_OFF + m * 100
                return wb[:, off:off + 100]

            # ---- kp[at] = WM_SCALE*(knn @ Wm + qproj bcast); hT = tanh(kp/S
            # + bqm).  fp8 DoubleRow, mc-outer: each round gated by its own
            # knnT8 piece DMA so the PE never builds a queued backlog ----
            hT = [sb.tile([128, NCD], bf16, tag=f"hT{at}", name=f"hTt{at}")
                  for at in range(2)]
            kp = [kpp.tile([128, NCD], f32, tag=f"kp{at}", name=f"kpt{at}")
                  for at in range(2)]
            # qproj fold goes FIRST (needs only wbA+Bmat -> overlaps knnT8 DMA)
            for at in range(2):
                for half in range(2):
                    sl = slice(half * 512, (half + 1) * 512)
                    off = QPB_OFF + at * 128
                    nc.tensor.matmul(
                        kp[at][:, sl], lhsT=wb[:BROWS, off:off + 128],
                        rhs=Bmat[:, sl], start=True, stop=False,
                        skip_group_check=True)
            sc_ps = ps1.tile([128, 8], f32, tag="small2")
            for half in range(2):
                sl = slice(half * 512, (half + 1) * 512)
                for mc in range(2):
                    for at in range(2):
                        nc.tensor.matmul(
                            kp[at][:, sl],
                            lhsT=wm8v[:, mc, :, at * 128:(at + 1) * 128],
                            rhs=kt8v[:, mc, :, sl],
                            start=False, stop=(mc == 1),
                            perf_mode=mybir.MatmulPerfMode.DoubleRow,
                            skip_group_check=True)
                for at in range(2):
                    nc.scalar.activation(hT[at][:, sl], kp[at][:, sl],
                                         mybir.ActivationFunctionType.Tanh,
                                         bias=bqm[:, at:at + 1],
                                         scale=1.0 / WM_SCALE)
                # scoresT for this half's cand tiles
                for t in range(half * 4, half * 4 + 4):
                    for at in range(2):
                        nc.tensor.matmul(
                            sc_ps[:, t:t + 1],
                            lhsT=hT[at][:, t * 128:(t + 1) * 128],
                            rhs=wb[:, WS_OFF + at:WS_OFF + at + 1],
                            start=(at == 0), stop=(at == 1))
            e_sb = sb.tile([128, 8], f32, tag="e_sb")
            nc.scalar.activation(e_sb[:], sc_ps[:],
                                 mybir.ActivationFunctionType.Exp)

            # ---- block softmax weights w2 [128, 256]: w2[p, t*32+b] ----
            w2 = sb.tile([128, 256], bf16, tag="w2")
            for t in range(8):
                nc.vector.tensor_scalar_mul(
                    w2[:, t * 32:(t + 1) * 32],
                    mw[:, 28 - 4 * t:60 - 4 * t],
                    e_sb[:, t:t + 1])

            # ---- den, attT (unscaled), classifier ----
            den_t = ps1.tile([128, 8], f32, tag="small2")
            den_ps = den_t[:BROWS, :1]
            for t in range(8):
                nc.tensor.matmul(den_ps, lhsT=w2[:, t * 32:(t + 1) * 32],
                                 rhs=ones1[:], start=(t == 0), stop=(t == 7))
            rden = sb.tile([BROWS, 1], f32, tag="rden")
            nc.vector.reciprocal(rden[:], den_ps)

            attT_ps = ps1.tile([128, 128], f32, tag="attT")
            for dc in range(4):
                for t in range(8):
                    nc.tensor.matmul(
                        attT_ps[:, dc * 32:(dc + 1) * 32],
                        lhsT=knn[:, t * D + dc * 128:t * D + (dc + 1) * 128],
                        rhs=w2[:, t * 32:(t + 1) * 32],
                        start=(t == 0), stop=(t == 7))
            attT = sb.tile([128, 128], bf16, tag="attT_sb")
            nc.scalar.copy(out=attT[:], in_=attT_ps[:])

            outa_ps = ps1.tile([BROWS, C], f32, tag="outa")
            for dc in range(4):
                nc.tensor.matmul(outa_ps[:], lhsT=attT[:, dc * 32:(dc + 1) * 32],
                                 rhs=wc(dc), start=(dc == 0), stop=(dc == 3))

            # out = outa * rden + outq  (fold 1/den at the end)
            out_sb = sb.tile([BROWS, C], f32, tag="out_sb")
            nc.vector.scalar_tensor_tensor(
                out=out_sb[:], in0=outa_ps[:], scalar=rden[:], in1=outq[:],
                op0=mybir.AluOpType.mult, op1=mybir.AluOpType.add)
            nc.sync.dma_start(out=out_d[:, :], in_=out_sb[:])
    nc.finalize()
    return nc


def _phase1_nc():
    global _PH1
    if _PH1 is None:
        _PH1 = _build_phase1()
    return _PH1


def _phase2_nc():
    global _PH2
    if _PH2 is None:
        _PH2 = _build_phase2()
    return _PH2


def kernel(query_feat, memory_keys, Wq, bq, Wm, bm, Ws, bs, Wc, bc):
    query_feat = np.asarray(query_feat, np.float32)
    memory_keys = np.asarray(memory_keys, np.float32)

    # ---- host prep: normalize keys, quantize to fp8, DoubleRow layout ----
    kn = np.sqrt((memory_keys ** 2).sum(axis=1))
    khat = memory_keys * (1.0 / kn)[:, None]
    khat_pad = np.zeros((NPAD, D), np.float32)
    khat_pad[:N] = khat
    k8 = khat_pad.astype(F8NP)

    q32 = np.maximum(query_feat, 0)
    q8 = q32.astype(F8NP)

    # per-row flag threshold tau_b = mu_b + z * sigma_b of the fp8 sims,
    # estimated from a uniform key sample (exact same arrays the PE sees)
    k8f = k8[:N].astype(np.float32)
    q8f = q8.astype(np.float32)
    samp = k8f[:: N // NSAMP][:NSAMP]
    sims_s = q8f @ samp.T                              # [B, NSAMP]
    mu = sims_s.mean(axis=1)
    sig = sims_s.std(axis=1)
    tau = (mu + ZTHRESH * sig).astype(np.float32)      # [B]
    # bias for ACT: Relu(sim + bias), bias = -tau, laid out [128, 2(qt)]
    bias = (-tau).reshape(2, 128).T.copy()             # [128, 2]

    # q8 DoubleRow layout: [128p, (mc two b)]
    q8arr = np.ascontiguousarray(
        q8.T.reshape(2, 2, 128, B).transpose(2, 0, 1, 3)).reshape(128, 1024)

    ph1 = _phase1_nc()
    in_maps = []
    for c in range(NC_CORES):
        sh = k8[c * SHARD:(c + 1) * SHARD]             # [12800, 512]
        arr = np.ascontiguousarray(
            sh.reshape(NCHUNK, CHUNK, 2, 2, 128).transpose(0, 4, 2, 3, 1)
        ).reshape(NCHUNK, 128, 4 * CHUNK)
        in_maps.append({"k8": arr, "q8": q8arr, "tau": bias})
    res1 = run_bass_kernel_spmd(ph1, in_maps, core_ids=list(range(NC_CORES)))

    # ---- host: scan flags, exact re-score, global top-32 ----
    flags = np.empty((B, NPAD), np.uint8)
    for c in range(NC_CORES):
        fl = res1.results[c]["fl"].view(np.uint8)      # [2, 128, SHARD]
        flags[:, c * SHARD:(c + 1) * SHARD] = fl.reshape(B, SHARD)
    flags &= 0x7F                                      # ignore sign bit of -0
    rows, cols = np.nonzero(flags)
    cnt = np.bincount(rows, minlength=B)
    Mx = max(int(cnt.max()), K)
    idxpad = np.zeros((B, Mx), np.int64)
    mask = np.arange(Mx)[None, :] < cnt[:, None]
    idxpad[mask] = cols
    ok = mask & (idxpad < N)

    qn = np.sqrt((q32 ** 2).sum(axis=1))
    safe = np.minimum(idxpad, N - 1)
    cand_keys = memory_keys[safe]                      # [B, Mx, D]
    dots = np.einsum("bd,bmd->bm", q32, cand_keys, optimize=True)
    cos = dots / np.maximum(qn[:, None] * kn[safe], np.float32(1e-8))
    cos[~ok] = -np.inf

    short = np.nonzero(ok.sum(axis=1) < K)[0]
    if short.size:                                     # statistical fallback
        sims_full = q32[short] @ memory_keys.T
        cos_full = sims_full / np.maximum(
            qn[short, None] * kn[None, :], np.float32(1e-8))
        top_f = np.argsort(-cos_full, axis=1, kind="stable")[:, :K]
    order = np.argsort(-cos, axis=1, kind="stable")[:, :K]
    top_idx = np.take_along_axis(safe, order, axis=1)  # [B, K]
    if short.size:
        top_idx[short] = top_f
    knn = memory_keys[top_idx]                         # [B, K, D]

    # ---- phase 2 (batch sharded, bf16 blob + fp8 scores path) ----
    ph2 = _phase2_nc()
    Wq_a = np.asarray(Wq, np.float32)
    Wm_a = np.asarray(Wm, np.float32)
    Ws_a = np.asarray(Ws, np.float32).reshape(A)
    Wc_a = np.asarray(Wc, np.float32)
    bqm = (np.asarray(bq, np.float32) + np.asarray(bm, np.float32))
    bqm_arr = np.ascontiguousarray(bqm.reshape(2, 128).T)          # [128, 2]

    wb = np.zeros((128, WB_COLS), np.float32)
    wb[:, WS_OFF:WS_OFF + 2] = Ws_a.reshape(2, 128).T
    wb[:, WC_OFF:WC_OFF + 400] = (
        Wc_a[D:].reshape(4, 128, C).transpose(1, 0, 2).reshape(128, 400))
    # wm8 [128p, (mc two a)] = WM_SCALE * Wm[mc*256+two*128+p, a]
    wm8_arr = np.ascontiguousarray(
        (WM_SCALE * Wm_a).reshape(2, 2, 128, A).transpose(2, 0, 1, 3)
    ).reshape(128, 1024).astype(F8NP)
    qproj_all = (q32 @ Wq_a) * WM_SCALE                # [B, A] host-side
    outq_all = q32 @ Wc_a[:D]                          # [B, C] host-side, exact
    # bmat [32, 1024]: B[b, cand] = 1 iff cand // 32 == b
    bmat_arr = (np.arange(NCD)[None, :] // K == np.arange(BROWS)[:, None]
                ).astype(BF16NP)

    in_maps2 = []
    for c in range(NC_CORES):
        rows2 = slice(c * BROWS, (c + 1) * BROWS)
        wb_c = wb.copy()
        wb_c[:BROWS, QPB_OFF:QPB_OFF + A] = qproj_all[rows2]
        knn_c = knn[rows2].reshape(NCD, D)             # [1024, 512]
        # knnT8 [128p, (mc two cand)] = knn_c[cand, mc*256+two*128+p]
        knnT8_c = np.ascontiguousarray(
            knn_c.T.reshape(2, 2, 128, NCD).transpose(2, 0, 1, 3)
        ).reshape(128, 4 * NCD).astype(F8NP)
        knn_l = np.ascontiguousarray(
            knn_c.reshape(8, 128, D).transpose(1, 0, 2).reshape(128, 8 * D))
        in_maps2.append({
            "wb": wb_c.astype(BF16NP),
            "bmat": bmat_arr,
            "wm8": wm8_arr,
            "bqm": bqm_arr,
            "outq": np.ascontiguousarray(outq_all[rows2]),
            "knnT8": knnT8_c,
            "knn": knn_l.astype(BF16NP),
        })
    res2 = run_bass_kernel_spmd(ph2, in_maps2, core_ids=list(range(NC_CORES)))
    out = np.concatenate([res2.results[c]["out"] for c in range(NC_CORES)], axis=0)
    return (out + np.asarray(bc, np.float32)[None, :]).astype(np.float32)
